# revision 6
# baseline (speedup 1.0000x reference)
"""Trainium2 Bass kernel for the topk_masking problem.

Math (per mention m):
    S^T[w, c] = sum_d T[m, w, d] * E[m, c, d]          (tokens x cands scores)
    ts[w]     = max_c S^T[w, c], masked to NEG where token invalid
    tau       = 25th largest value of ts
    p[w]      = exp(ts - max) * [ts >= tau] / Z        (softmax over top-25)
    vals[c]   = sum_w S^T[w, c] * p[w]                 ( == E @ (T^T @ p) )

This avoids the gather (take_along_axis) entirely: tokens outside the top-25
get weight exactly ~0, so vals = S @ p reproduces the reference (B_diag1 and
B_diag2 are ones per the problem spec).

Data-parallel over mentions: 1024 mentions per core x 8 cores.
"""

import os
from contextlib import ExitStack

import numpy as np

import concourse.bass as bass
import concourse.bacc as bacc
import concourse.tile as tile
from concourse import mybir
from concourse.bass_utils import run_bass_kernel_spmd

F32 = mybir.dt.float32

N_CORES = 8
NMENT = 8192
NCAND = 30
D = 300
WIN = 100
TOPK = 25
NEG = -1.0e10
MINVAL = -3.0e38  # zap value for match_replace; below any real score incl NEG

G = 16          # mentions per PSUM accumulation group (16*30=480 fp32 <= 512)
SB = 128        # mentions per superblock (topk/softmax batch; 128 partitions)
KC = 3          # contraction chunks over d
KD = 100        # chunk size (3*100 = 300 = D)

LAST_RESULT = None


def _build_nc(n_mentions):
    nsb = n_mentions // SB
    gpb = SB // G           # groups per superblock
    nblk = n_mentions // G  # total groups

    nc = bacc.Bacc(None)
    t_d = nc.declare_dram_parameter("t", [nblk, KD, KC, G, WIN], F32, isOutput=False)
    e_d = nc.declare_dram_parameter("e", [nblk, KD, KC, G, NCAND], F32, isOutput=False)
    im_d = nc.declare_dram_parameter("im", [nsb, SB, WIN], mybir.dt.uint8, isOutput=False)
    id_d = nc.declare_dram_parameter("ident", [128, 128], F32, isOutput=False)
    o_d = nc.declare_dram_parameter("o", [nsb, SB * NCAND], F32, isOutput=True)

    with tile.TileContext(nc) as tc, ExitStack() as ctx:
        singles = ctx.enter_context(tc.tile_pool(name="singles", bufs=1))
        tpool = ctx.enter_context(tc.tile_pool(name="tpool", bufs=3))
        epool = ctx.enter_context(tc.tile_pool(name="epool", bufs=3))
        spool = ctx.enter_context(tc.tile_pool(name="spool", bufs=2))
        scpool = ctx.enter_context(tc.tile_pool(name="scpool", bufs=3))
        small = ctx.enter_context(tc.tile_pool(name="small", bufs=2))
        vpool = ctx.enter_context(tc.tile_pool(name="vpool", bufs=2))
        ps_s = ctx.enter_context(tc.tile_pool(name="ps_s", bufs=3, space="PSUM"))
        ps_t = ctx.enter_context(tc.tile_pool(name="ps_t", bufs=1, space="PSUM"))
        ps_v = ctx.enter_context(tc.tile_pool(name="ps_v", bufs=2, space="PSUM"))

        ident = singles.tile([128, 128], F32)
        nc.sync.dma_start(out=ident, in_=id_d[:, :])
        ones = singles.tile([KD, 1], F32)
        nc.vector.memset(ones, 1.0)
        negt = singles.tile([SB, WIN], F32)
        nc.vector.memset(negt, NEG)

        X = mybir.AxisListType.X

        for sb in range(nsb):
            invm = small.tile([SB, WIN], mybir.dt.uint8, tag="invm")
            nc.sync.dma_start(out=invm, in_=im_d[sb])
            ts_stage = small.tile([WIN, SB], F32, tag="ts_stage")
            st_sb = spool.tile([WIN, SB, NCAND], F32, tag="st")

            for g in range(gpb):
                blk = sb * gpb + g
                tt = tpool.tile([KD, KC, G, WIN], F32, tag="tt")
                nc.sync.dma_start(out=tt, in_=t_d[blk])
                et = epool.tile([KD, KC, G, NCAND], F32, tag="et")
                nc.sync.dma_start(out=et, in_=e_d[blk])

                sps = ps_s.tile([WIN, G, NCAND], F32, tag="sps")
                for b in range(G):
                    for k in range(KC):
                        nc.tensor.matmul(
                            sps[:, b, :],
                            lhsT=tt[:, k, b, :],
                            rhs=et[:, k, b, :],
                            start=(k == 0),
                            stop=(k == KC - 1),
                        )
                nc.vector.reduce_max(ts_stage[:, g * G:(g + 1) * G], sps, axis=X)
                nc.scalar.copy(st_sb[:, g * G:(g + 1) * G, :], sps)

            # ---- superblock stage: mask, topk threshold, softmax ----
            tst = ps_t.tile([SB, WIN], F32, tag="tst")
            nc.tensor.transpose(tst, ts_stage, ident[0:WIN, 0:WIN])
            ts = small.tile([SB, WIN], F32, tag="ts")
            nc.scalar.copy(ts, tst)
            nc.vector.copy_predicated(ts, invm, negt)

            mx1 = small.tile([SB, 8], F32, tag="mx1")
            work = small.tile([SB, WIN], F32, tag="work")
            nc.vector.max(mx1, ts)
            nc.vector.match_replace(
                out=work, in_to_replace=mx1, in_values=ts, imm_value=MINVAL)
            nzap = (TOPK - 1) // 8  # 3 rounds of zapping -> 24 zapped
            for r in range(1, nzap):
                mxr = small.tile([SB, 8], F32, tag=f"mx{r + 1}")
                nc.vector.max(mxr, work)
                nc.vector.match_replace(
                    out=work, in_to_replace=mxr, in_values=work, imm_value=MINVAL)
            mxf = small.tile([SB, 8], F32, tag="mxf")
            nc.vector.max(mxf, work)
            kidx = (TOPK - 1) % 8
            tau = mxf[:, kidx:kidx + 1]

            shifted = small.tile([SB, WIN], F32, tag="shifted")
            nc.vector.tensor_scalar(
                shifted, ts, mx1[:, 0:1], None, op0=mybir.AluOpType.subtract)
            nc.vector.tensor_scalar_max(shifted, shifted, -80.0)
            ex = small.tile([SB, WIN], F32, tag="ex")
            nc.scalar.activation(ex, shifted, mybir.ActivationFunctionType.Exp)
            msk = small.tile([SB, WIN], F32, tag="msk")
            nc.vector.tensor_scalar(
                msk, ts, tau, None, op0=mybir.AluOpType.is_ge)
            nc.vector.tensor_mul(ex, ex, msk)
            z = small.tile([SB, 1], F32, tag="z")
            nc.vector.reduce_sum(z, ex, axis=X)
            rz = small.tile([SB, 1], F32, tag="rz")
            nc.vector.reciprocal(rz, z)
            p = small.tile([SB, WIN], F32, tag="p")
            nc.vector.tensor_scalar_mul(p, ex, rz)

            ptps = ps_t.tile([WIN, SB], F32, tag="ptps")
            nc.tensor.transpose(ptps, p, ident)
            pt = small.tile([WIN, SB], F32, tag="pt")
            nc.scalar.copy(pt, ptps)

            # ---- vals = S^T scaled by p, column-summed via ones matmul ----
            vals = vpool.tile([1, SB * NCAND], F32, tag="vals")
            for g in range(gpb):
                sc = scpool.tile([WIN, G, NCAND], F32, tag="sc")
                psl = pt[:, g * G:(g + 1) * G]
                pbc = bass.AP(
                    tensor=psl.tensor,
                    offset=psl.offset,
                    ap=[psl.ap[0], psl.ap[1], [0, NCAND]],
                )
                nc.vector.tensor_mul(sc, st_sb[:, g * G:(g + 1) * G, :], pbc)
                vps = ps_v.tile([1, G * NCAND], F32, tag="vps")
                nc.tensor.matmul(vps, lhsT=ones, rhs=sc, start=True, stop=True)
                nc.scalar.copy(vals[:, g * G * NCAND:(g + 1) * G * NCAND], vps)
            nc.sync.dma_start(out=o_d[sb:sb + 1], in_=vals)

    nc.compile()
    return nc


def _prep_core(E, T, M):
    """Host-side layout: per-group chunked transposes.

    t[blk, p, k, b, w] = T[blk*G + b, w, k*KD + p]
    e[blk, p, k, b, c] = E[blk*G + b, c, k*KD + p]
    """
    n = E.shape[0]
    nblk = n // G
    nsb = n // SB
    Tt = np.ascontiguousarray(T.transpose(0, 2, 1))  # (n, D, WIN)
    t = np.ascontiguousarray(
        Tt.reshape(nblk, G, KC, KD, WIN).transpose(0, 3, 2, 1, 4))
    Et = np.ascontiguousarray(E.transpose(0, 2, 1))  # (n, D, NCAND)
    e = np.ascontiguousarray(
        Et.reshape(nblk, G, KC, KD, NCAND).transpose(0, 3, 2, 1, 4))
    im = np.ascontiguousarray(
        (~M).astype(np.uint8).reshape(nsb, SB, WIN))
    return {
        "t": t,
        "e": e,
        "im": im,
        "ident": np.eye(128, dtype=np.float32),
    }


def _install_ntff_shim():
    """The image's antenv package lacks axon_hooks; provide it so
    run_bass_kernel_spmd(trace=True) can capture NTFF profiles."""
    import sys
    import types
    try:
        from antenv import axon_hooks  # noqa: F401
        return
    except ImportError:
        pass
    try:
        import antenv
        mod = types.ModuleType("antenv.axon_hooks")
        mod._hook = None

        def set_axon_ntff_profile_hook(h):
            mod._hook = h

        def get_axon_ntff_profile_hook():
            return mod._hook

        mod.set_axon_ntff_profile_hook = set_axon_ntff_profile_hook
        mod.get_axon_ntff_profile_hook = get_axon_ntff_profile_hook
        sys.modules["antenv.axon_hooks"] = mod
        antenv.axon_hooks = mod
        if "/root/.axon_site" not in sys.path:
            sys.path.insert(0, "/root/.axon_site")
        from trn_agent_boot.trn_boot import _ntff_profile_via_ctypes
        hook = _ntff_profile_via_ctypes("/opt/axon/libaxon_pjrt.so")
        if hook is not None:
            set_axon_ntff_profile_hook(hook)
    except Exception as exc:  # tracing is best-effort
        print("ntff shim failed:", exc)


def kernel(embeddings, tokenEmbeddings, tokenMaskss, B_diag1, B_diag2):
    global LAST_RESULT
    E = np.ascontiguousarray(np.asarray(embeddings, dtype=np.float32))
    T = np.ascontiguousarray(np.asarray(tokenEmbeddings, dtype=np.float32))
    M = np.asarray(tokenMaskss).astype(bool)
    b1 = np.asarray(B_diag1, dtype=np.float32)
    b2 = np.asarray(B_diag2, dtype=np.float32)
    # Fold diag scales into the operands (spec: both are ones; this keeps
    # generality for B_diag2 and B_diag1 == B_diag2 cases).
    if not np.all(b2 == 1.0):
        T = T * b2[None, None, :]
    n = E.shape[0]
    per = n // N_CORES

    nc = _build_nc(per)
    in_maps = []
    for c in range(N_CORES):
        sl = slice(c * per, (c + 1) * per)
        in_maps.append(_prep_core(E[sl], T[sl], M[sl]))

    trace = os.environ.get("KTRACE", "0") == "1"
    if trace:
        _install_ntff_shim()
    res = run_bass_kernel_spmd(
        nc, in_maps, core_ids=list(range(N_CORES)), trace=trace)
    LAST_RESULT = res
    out = np.concatenate(
        [r["o"].reshape(per, NCAND) for r in res.results], axis=0)
    return out.astype(np.float32)


# revision 7
# speedup vs baseline: 2.4205x; 2.4205x over previous
"""Trainium2 Bass kernel for the topk_masking problem.

Math (per mention m):
    S^T[w, c] = sum_d T[m, w, d] * E[m, c, d]          (tokens x cands scores)
    ts[w]     = max_c S^T[w, c], masked to NEG where token invalid
    tau       = 25th largest value of ts
    p[w]      = exp(ts - max) * [ts >= tau] / Z        (softmax over top-25)
    vals[c]   = sum_w S^T[w, c] * p[w]                 ( == E @ (T^T @ p) )

This avoids the gather (take_along_axis) entirely: tokens outside the top-25
get weight exactly ~0, so vals = S @ p reproduces the reference (B_diag1 and
B_diag2 are ones per the problem spec).

Data-parallel over mentions: 1024 mentions per core x 8 cores.
"""

import os
from contextlib import ExitStack

import numpy as np

import concourse.bass as bass
import concourse.bacc as bacc
import concourse.tile as tile
from concourse import mybir
from concourse.bass_utils import run_bass_kernel_spmd

F32 = mybir.dt.float32
F16 = mybir.dt.float16

N_CORES = 8
NMENT = 8192
NCAND = 30
D = 300
WIN = 100
TOPK = 25
NEG = -1.0e10
MINVAL = -3.0e38  # zap value for match_replace; below any real score incl NEG

G = 16          # mentions per PSUM accumulation group (16*30=480 fp32 <= 512)
SB = 128        # mentions per superblock (topk/softmax batch; 128 partitions)
KC = 3          # contraction chunks over d
KD = 100        # chunk size (3*100 = 300 = D)

LAST_RESULT = None


def _build_nc(n_mentions):
    nsb = n_mentions // SB
    gpb = SB // G           # groups per superblock
    nblk = n_mentions // G  # total groups

    nc = bacc.Bacc(None)
    t_d = nc.declare_dram_parameter("t", [nblk, KD, KC, G, WIN], F16, isOutput=False)
    e_d = nc.declare_dram_parameter("e", [nblk, KD, KC, G, NCAND], F16, isOutput=False)
    im_d = nc.declare_dram_parameter("im", [nsb, SB, WIN], mybir.dt.uint8, isOutput=False)
    id_d = nc.declare_dram_parameter("ident", [128, 128], F32, isOutput=False)
    o_d = nc.declare_dram_parameter("o", [nsb, SB * NCAND], F32, isOutput=True)

    with tile.TileContext(nc) as tc, ExitStack() as ctx:
        singles = ctx.enter_context(tc.tile_pool(name="singles", bufs=1))
        tpool = ctx.enter_context(tc.tile_pool(name="tpool", bufs=4))
        epool = ctx.enter_context(tc.tile_pool(name="epool", bufs=4))
        spool = ctx.enter_context(tc.tile_pool(name="spool", bufs=2))
        scpool = ctx.enter_context(tc.tile_pool(name="scpool", bufs=3))
        small = ctx.enter_context(tc.tile_pool(name="small", bufs=2))
        vpool = ctx.enter_context(tc.tile_pool(name="vpool", bufs=2))
        ps_s = ctx.enter_context(tc.tile_pool(name="ps_s", bufs=3, space="PSUM"))
        ps_t = ctx.enter_context(tc.tile_pool(name="ps_t", bufs=1, space="PSUM"))
        ps_v = ctx.enter_context(tc.tile_pool(name="ps_v", bufs=2, space="PSUM"))

        ident = singles.tile([128, 128], F32)
        nc.sync.dma_start(out=ident, in_=id_d[:, :])
        ones = singles.tile([KD, 1], F32)
        nc.vector.memset(ones, 1.0)
        negt = singles.tile([SB, WIN], F32)
        nc.vector.memset(negt, NEG)

        X = mybir.AxisListType.X

        for sb in range(nsb):
            invm = small.tile([SB, WIN], mybir.dt.uint8, tag="invm")
            nc.sync.dma_start(out=invm, in_=im_d[sb])
            ts_stage = small.tile([WIN, SB], F32, tag="ts_stage")
            st_sb = spool.tile([WIN, SB, NCAND], F32, tag="st")

            for g in range(gpb):
                blk = sb * gpb + g
                tt = tpool.tile([KD, KC, G, WIN], F16, tag="tt")
                nc.sync.dma_start(out=tt, in_=t_d[blk])
                et = epool.tile([KD, KC, G, NCAND], F16, tag="et")
                nc.scalar.dma_start(out=et, in_=e_d[blk])

                sps = ps_s.tile([WIN, G, NCAND], F32, tag="sps")
                for b in range(G):
                    for k in range(KC):
                        nc.tensor.matmul(
                            sps[:, b, :],
                            lhsT=tt[:, k, b, :],
                            rhs=et[:, k, b, :],
                            start=(k == 0),
                            stop=(k == KC - 1),
                        )
                nc.vector.reduce_max(ts_stage[:, g * G:(g + 1) * G], sps, axis=X)
                nc.scalar.copy(st_sb[:, g * G:(g + 1) * G, :], sps)

            # ---- superblock stage: mask, topk threshold, softmax ----
            tst = ps_t.tile([SB, WIN], F32, tag="tst")
            nc.tensor.transpose(tst, ts_stage, ident[0:WIN, 0:WIN])
            ts = small.tile([SB, WIN], F32, tag="ts")
            nc.scalar.copy(ts, tst)
            nc.vector.copy_predicated(ts, invm, negt)

            mx1 = small.tile([SB, 8], F32, tag="mx1")
            work = small.tile([SB, WIN], F32, tag="work")
            nc.vector.max(mx1, ts)
            nc.vector.match_replace(
                out=work, in_to_replace=mx1, in_values=ts, imm_value=MINVAL)
            nzap = (TOPK - 1) // 8  # 3 rounds of zapping -> 24 zapped
            for r in range(1, nzap):
                mxr = small.tile([SB, 8], F32, tag=f"mx{r + 1}")
                nc.vector.max(mxr, work)
                nc.vector.match_replace(
                    out=work, in_to_replace=mxr, in_values=work, imm_value=MINVAL)
            mxf = small.tile([SB, 8], F32, tag="mxf")
            nc.vector.max(mxf, work)
            kidx = (TOPK - 1) % 8
            tau = mxf[:, kidx:kidx + 1]

            shifted = small.tile([SB, WIN], F32, tag="shifted")
            nc.vector.tensor_scalar(
                shifted, ts, mx1[:, 0:1], None, op0=mybir.AluOpType.subtract)
            nc.vector.tensor_scalar_max(shifted, shifted, -80.0)
            ex = small.tile([SB, WIN], F32, tag="ex")
            nc.scalar.activation(ex, shifted, mybir.ActivationFunctionType.Exp)
            msk = small.tile([SB, WIN], F32, tag="msk")
            nc.vector.tensor_scalar(
                msk, ts, tau, None, op0=mybir.AluOpType.is_ge)
            nc.vector.tensor_mul(ex, ex, msk)
            z = small.tile([SB, 1], F32, tag="z")
            nc.vector.reduce_sum(z, ex, axis=X)
            rz = small.tile([SB, 1], F32, tag="rz")
            nc.vector.reciprocal(rz, z)
            p = small.tile([SB, WIN], F32, tag="p")
            nc.vector.tensor_scalar_mul(p, ex, rz)

            ptps = ps_t.tile([WIN, SB], F32, tag="ptps")
            nc.tensor.transpose(ptps, p, ident)
            pt = small.tile([WIN, SB], F32, tag="pt")
            nc.scalar.copy(pt, ptps)

            # ---- vals = S^T scaled by p, column-summed via ones matmul ----
            vals = vpool.tile([1, SB * NCAND], F32, tag="vals")
            for g in range(gpb):
                sc = scpool.tile([WIN, G, NCAND], F32, tag="sc")
                psl = pt[:, g * G:(g + 1) * G]
                pbc = bass.AP(
                    tensor=psl.tensor,
                    offset=psl.offset,
                    ap=[psl.ap[0], psl.ap[1], [0, NCAND]],
                )
                nc.vector.tensor_mul(sc, st_sb[:, g * G:(g + 1) * G, :], pbc)
                vps = ps_v.tile([1, G * NCAND], F32, tag="vps")
                nc.tensor.matmul(vps, lhsT=ones, rhs=sc, start=True, stop=True)
                nc.scalar.copy(vals[:, g * G * NCAND:(g + 1) * G * NCAND], vps)
            nc.sync.dma_start(out=o_d[sb:sb + 1], in_=vals)

    nc.compile()
    return nc


def _prep_core(E, T, M):
    """Host-side layout: per-group chunked transposes.

    t[blk, p, k, b, w] = T[blk*G + b, w, k*KD + p]
    e[blk, p, k, b, c] = E[blk*G + b, c, k*KD + p]
    """
    n = E.shape[0]
    nblk = n // G
    nsb = n // SB
    Tt = np.ascontiguousarray(T.transpose(0, 2, 1))  # (n, D, WIN)
    t = np.ascontiguousarray(
        Tt.reshape(nblk, G, KC, KD, WIN).transpose(0, 3, 2, 1, 4).astype(np.float16))
    Et = np.ascontiguousarray(E.transpose(0, 2, 1))  # (n, D, NCAND)
    e = np.ascontiguousarray(
        Et.reshape(nblk, G, KC, KD, NCAND).transpose(0, 3, 2, 1, 4).astype(np.float16))
    im = np.ascontiguousarray(
        (~M).astype(np.uint8).reshape(nsb, SB, WIN))
    return {
        "t": t,
        "e": e,
        "im": im,
        "ident": np.eye(128, dtype=np.float32),
    }


def _install_ntff_shim():
    """The image's antenv package lacks axon_hooks; provide it so
    run_bass_kernel_spmd(trace=True) can capture NTFF profiles."""
    import sys
    import types
    try:
        from antenv import axon_hooks  # noqa: F401
        return
    except ImportError:
        pass
    try:
        import antenv
        mod = types.ModuleType("antenv.axon_hooks")
        mod._hook = None

        def set_axon_ntff_profile_hook(h):
            mod._hook = h

        def get_axon_ntff_profile_hook():
            return mod._hook

        mod.set_axon_ntff_profile_hook = set_axon_ntff_profile_hook
        mod.get_axon_ntff_profile_hook = get_axon_ntff_profile_hook
        sys.modules["antenv.axon_hooks"] = mod
        antenv.axon_hooks = mod
        if "/root/.axon_site" not in sys.path:
            sys.path.insert(0, "/root/.axon_site")
        from trn_agent_boot.trn_boot import _ntff_profile_via_ctypes
        hook = _ntff_profile_via_ctypes("/opt/axon/libaxon_pjrt.so")
        if hook is not None:
            set_axon_ntff_profile_hook(hook)
    except Exception as exc:  # tracing is best-effort
        print("ntff shim failed:", exc)


def kernel(embeddings, tokenEmbeddings, tokenMaskss, B_diag1, B_diag2):
    global LAST_RESULT
    E = np.ascontiguousarray(np.asarray(embeddings, dtype=np.float32))
    T = np.ascontiguousarray(np.asarray(tokenEmbeddings, dtype=np.float32))
    M = np.asarray(tokenMaskss).astype(bool)
    b1 = np.asarray(B_diag1, dtype=np.float32)
    b2 = np.asarray(B_diag2, dtype=np.float32)
    # Fold diag scales into the operands (spec: both are ones; this keeps
    # generality for B_diag2 and B_diag1 == B_diag2 cases).
    if not np.all(b2 == 1.0):
        T = T * b2[None, None, :]
    n = E.shape[0]
    per = n // N_CORES

    nc = _build_nc(per)
    in_maps = []
    for c in range(N_CORES):
        sl = slice(c * per, (c + 1) * per)
        in_maps.append(_prep_core(E[sl], T[sl], M[sl]))

    trace = os.environ.get("KTRACE", "0") == "1"
    if trace:
        _install_ntff_shim()
    res = run_bass_kernel_spmd(
        nc, in_maps, core_ids=list(range(N_CORES)), trace=trace)
    LAST_RESULT = res
    out = np.concatenate(
        [r["o"].reshape(per, NCAND) for r in res.results], axis=0)
    return out.astype(np.float32)


# revision 9
# speedup vs baseline: 3.0394x; 1.2557x over previous
"""Trainium2 Bass kernel for the topk_masking problem.

Math (per mention m):
    S^T[w, c] = sum_d T[m, w, d] * E[m, c, d]          (tokens x cands scores)
    ts[w]     = max_c S^T[w, c], masked to NEG where token invalid
    tau       = 25th largest value of ts
    p[w]      = exp(ts - max) * [ts >= tau] / Z        (softmax over top-25)
    vals[c]   = sum_w S^T[w, c] * p[w]                 ( == E @ (T^T @ p) )

This avoids the gather (take_along_axis) entirely: tokens outside the top-25
get weight exactly ~0, so vals = S @ p reproduces the reference (B_diag1 and
B_diag2 are ones per the problem spec).

Data-parallel over mentions: 1024 mentions per core x 8 cores.
"""

import os
from contextlib import ExitStack

import numpy as np

import concourse.bass as bass
import concourse.bacc as bacc
import concourse.tile as tile
from concourse import mybir
from concourse.bass_utils import run_bass_kernel_spmd

F32 = mybir.dt.float32
F16 = mybir.dt.float16

N_CORES = 8
NMENT = 8192
NCAND = 30
D = 300
WIN = 100
TOPK = 25
NEG = -1.0e10
MINVAL = -3.0e38  # zap value for match_replace; below any real score incl NEG

G = 16          # mentions per PSUM accumulation group (16*30=480 fp32 <= 512)
SB = 128        # mentions per superblock (topk/softmax batch; 128 partitions)
KC = 3          # contraction chunks over d
KD = 128        # chunk size (3*128 = 384 = D zero-padded); 128 partitions
DPAD = KC * KD  # 384

LAST_RESULT = None


def _build_nc(n_mentions):
    nsb = n_mentions // SB
    gpb = SB // G           # groups per superblock
    nblk = n_mentions // G  # total groups

    nc = bacc.Bacc(None)
    t_d = nc.declare_dram_parameter("t", [nblk, KD, KC, G, WIN], F16, isOutput=False)
    e_d = nc.declare_dram_parameter("e", [nblk, KD, KC, G, NCAND], F16, isOutput=False)
    im_d = nc.declare_dram_parameter("im", [nsb, SB, WIN], mybir.dt.uint8, isOutput=False)
    id_d = nc.declare_dram_parameter("ident", [128, 128], F32, isOutput=False)
    o_d = nc.declare_dram_parameter("o", [nsb, SB * NCAND], F32, isOutput=True)

    with tile.TileContext(nc) as tc, ExitStack() as ctx:
        singles = ctx.enter_context(tc.tile_pool(name="singles", bufs=1))
        tpool = ctx.enter_context(tc.tile_pool(name="tpool", bufs=4))
        epool = ctx.enter_context(tc.tile_pool(name="epool", bufs=4))
        spool = ctx.enter_context(tc.tile_pool(name="spool", bufs=2))
        scpool = ctx.enter_context(tc.tile_pool(name="scpool", bufs=3))
        small = ctx.enter_context(tc.tile_pool(name="small", bufs=2))
        vpool = ctx.enter_context(tc.tile_pool(name="vpool", bufs=2))
        ps_s = ctx.enter_context(tc.tile_pool(name="ps_s", bufs=3, space="PSUM"))
        ps_t = ctx.enter_context(tc.tile_pool(name="ps_t", bufs=1, space="PSUM"))
        ps_v = ctx.enter_context(tc.tile_pool(name="ps_v", bufs=2, space="PSUM"))

        ident = singles.tile([128, 128], F32)
        nc.sync.dma_start(out=ident, in_=id_d[:, :])
        ones = singles.tile([WIN, 1], F32)
        nc.vector.memset(ones, 1.0)
        negt = singles.tile([SB, WIN], F32)
        nc.vector.memset(negt, NEG)

        X = mybir.AxisListType.X

        for sb in range(nsb):
            invm = small.tile([SB, WIN], mybir.dt.uint8, tag="invm")
            nc.sync.dma_start(out=invm, in_=im_d[sb])
            ts_stage = small.tile([WIN, SB], F32, tag="ts_stage")
            st_sb = spool.tile([WIN, SB, NCAND], F32, tag="st")

            for g in range(gpb):
                blk = sb * gpb + g
                tt = tpool.tile([KD, KC, G, WIN], F16, tag="tt")
                nc.sync.dma_start(out=tt, in_=t_d[blk])
                et = epool.tile([KD, KC, G, NCAND], F16, tag="et")
                nc.scalar.dma_start(out=et, in_=e_d[blk])

                sps = ps_s.tile([WIN, G, NCAND], F32, tag="sps")
                for b in range(G):
                    for k in range(KC):
                        nc.tensor.matmul(
                            sps[:, b, :],
                            lhsT=tt[:, k, b, :],
                            rhs=et[:, k, b, :],
                            start=(k == 0),
                            stop=(k == KC - 1),
                        )
                nc.vector.reduce_max(ts_stage[:, g * G:(g + 1) * G], sps, axis=X)
                nc.scalar.copy(st_sb[:, g * G:(g + 1) * G, :], sps)

            # ---- superblock stage: mask, topk threshold, softmax ----
            tst = ps_t.tile([SB, WIN], F32, tag="tst")
            nc.tensor.transpose(tst, ts_stage, ident[0:WIN, 0:WIN])
            ts = small.tile([SB, WIN], F32, tag="ts")
            nc.scalar.copy(ts, tst)
            nc.vector.copy_predicated(ts, invm, negt)

            mx1 = small.tile([SB, 8], F32, tag="mx1")
            work = small.tile([SB, WIN], F32, tag="work")
            nc.vector.max(mx1, ts)
            nc.vector.match_replace(
                out=work, in_to_replace=mx1, in_values=ts, imm_value=MINVAL)
            nzap = (TOPK - 1) // 8  # 3 rounds of zapping -> 24 zapped
            for r in range(1, nzap):
                mxr = small.tile([SB, 8], F32, tag=f"mx{r + 1}")
                nc.vector.max(mxr, work)
                nc.vector.match_replace(
                    out=work, in_to_replace=mxr, in_values=work, imm_value=MINVAL)
            mxf = small.tile([SB, 8], F32, tag="mxf")
            nc.vector.max(mxf, work)
            kidx = (TOPK - 1) % 8
            tau = mxf[:, kidx:kidx + 1]

            shifted = small.tile([SB, WIN], F32, tag="shifted")
            nc.vector.tensor_scalar(
                shifted, ts, mx1[:, 0:1], None, op0=mybir.AluOpType.subtract)
            nc.vector.tensor_scalar_max(shifted, shifted, -80.0)
            ex = small.tile([SB, WIN], F32, tag="ex")
            nc.scalar.activation(ex, shifted, mybir.ActivationFunctionType.Exp)
            msk = small.tile([SB, WIN], F32, tag="msk")
            nc.vector.tensor_scalar(
                msk, ts, tau, None, op0=mybir.AluOpType.is_ge)
            nc.vector.tensor_mul(ex, ex, msk)
            z = small.tile([SB, 1], F32, tag="z")
            nc.vector.reduce_sum(z, ex, axis=X)
            rz = small.tile([SB, 1], F32, tag="rz")
            nc.vector.reciprocal(rz, z)
            p = small.tile([SB, WIN], F32, tag="p")
            nc.vector.tensor_scalar_mul(p, ex, rz)

            ptps = ps_t.tile([WIN, SB], F32, tag="ptps")
            nc.tensor.transpose(ptps, p, ident)
            pt = small.tile([WIN, SB], F32, tag="pt")
            nc.scalar.copy(pt, ptps)

            # ---- vals = S^T scaled by p, column-summed via ones matmul ----
            vals = vpool.tile([1, SB * NCAND], F32, tag="vals")
            for g in range(gpb):
                sc = scpool.tile([WIN, G, NCAND], F32, tag="sc")
                psl = pt[:, g * G:(g + 1) * G]
                pbc = bass.AP(
                    tensor=psl.tensor,
                    offset=psl.offset,
                    ap=[psl.ap[0], psl.ap[1], [0, NCAND]],
                )
                nc.vector.tensor_mul(sc, st_sb[:, g * G:(g + 1) * G, :], pbc)
                vps = ps_v.tile([1, G * NCAND], F32, tag="vps")
                nc.tensor.matmul(vps, lhsT=ones, rhs=sc, start=True, stop=True)
                nc.scalar.copy(vals[:, g * G * NCAND:(g + 1) * G * NCAND], vps)
            nc.sync.dma_start(out=o_d[sb:sb + 1], in_=vals)

    nc.compile()
    return nc


def _prep_core(E, T, M):
    """Host-side layout: per-group chunked transposes.

    t[blk, p, k, b, w] = T[blk*G + b, w, k*KD + p]
    e[blk, p, k, b, c] = E[blk*G + b, c, k*KD + p]
    """
    n = E.shape[0]
    nblk = n // G
    nsb = n // SB
    Tt = np.zeros((n, DPAD, WIN), dtype=np.float16)
    Tt[:, :D, :] = T.transpose(0, 2, 1)
    t = np.ascontiguousarray(
        Tt.reshape(nblk, G, KC, KD, WIN).transpose(0, 3, 2, 1, 4))
    Et = np.zeros((n, DPAD, NCAND), dtype=np.float16)
    Et[:, :D, :] = E.transpose(0, 2, 1)
    e = np.ascontiguousarray(
        Et.reshape(nblk, G, KC, KD, NCAND).transpose(0, 3, 2, 1, 4))
    im = np.ascontiguousarray(
        (~M).astype(np.uint8).reshape(nsb, SB, WIN))
    return {
        "t": t,
        "e": e,
        "im": im,
        "ident": np.eye(128, dtype=np.float32),
    }


def _install_ntff_shim():
    """The image's antenv package lacks axon_hooks; provide it so
    run_bass_kernel_spmd(trace=True) can capture NTFF profiles."""
    import sys
    import types
    try:
        from antenv import axon_hooks  # noqa: F401
        return
    except ImportError:
        pass
    try:
        import antenv
        mod = types.ModuleType("antenv.axon_hooks")
        mod._hook = None

        def set_axon_ntff_profile_hook(h):
            mod._hook = h

        def get_axon_ntff_profile_hook():
            return mod._hook

        mod.set_axon_ntff_profile_hook = set_axon_ntff_profile_hook
        mod.get_axon_ntff_profile_hook = get_axon_ntff_profile_hook
        sys.modules["antenv.axon_hooks"] = mod
        antenv.axon_hooks = mod
        if "/root/.axon_site" not in sys.path:
            sys.path.insert(0, "/root/.axon_site")
        from trn_agent_boot.trn_boot import _ntff_profile_via_ctypes
        hook = _ntff_profile_via_ctypes("/opt/axon/libaxon_pjrt.so")
        if hook is not None:
            set_axon_ntff_profile_hook(hook)
    except Exception as exc:  # tracing is best-effort
        print("ntff shim failed:", exc)


def kernel(embeddings, tokenEmbeddings, tokenMaskss, B_diag1, B_diag2):
    global LAST_RESULT
    E = np.ascontiguousarray(np.asarray(embeddings, dtype=np.float32))
    T = np.ascontiguousarray(np.asarray(tokenEmbeddings, dtype=np.float32))
    M = np.asarray(tokenMaskss).astype(bool)
    b1 = np.asarray(B_diag1, dtype=np.float32)
    b2 = np.asarray(B_diag2, dtype=np.float32)
    # Fold diag scales into the operands (spec: both are ones; this keeps
    # generality for B_diag2 and B_diag1 == B_diag2 cases).
    if not np.all(b2 == 1.0):
        T = T * b2[None, None, :]
    n = E.shape[0]
    per = n // N_CORES

    nc = _build_nc(per)
    in_maps = []
    for c in range(N_CORES):
        sl = slice(c * per, (c + 1) * per)
        in_maps.append(_prep_core(E[sl], T[sl], M[sl]))

    trace = os.environ.get("KTRACE", "0") == "1"
    if trace:
        _install_ntff_shim()
    res = run_bass_kernel_spmd(
        nc, in_maps, core_ids=list(range(N_CORES)), trace=trace)
    LAST_RESULT = res
    out = np.concatenate(
        [r["o"].reshape(per, NCAND) for r in res.results], axis=0)
    return out.astype(np.float32)
